# revision 31
# baseline (speedup 1.0000x reference)
"""MultiHeadCrossAttention on 8 TRN2 NeuronCores.

Sharding: core c -> batch b = c//2, head-group g = c%2 (8 heads, 512 out dims).
Each core computes its head-group's Q/K/V projections, attention, and a
partial out-projection (Wo columns restricted to its head-group). Host sums
the two partials per batch and adds bo.

Fully software-pipelined schedule. Phase E (attention) is ACT-bound
(~8.9us/iter of exp vs ~7.2us of PE matmuls), so all projection work that
is not needed to START attention is interleaved into E's spare PE slots:

  prologue: V proj (gates every PV chunk), Q proj m=0, K proj m=0
  E iters (h-major, n-outer), pumping a feed of deferred matmuls between
    score chunks: K proj m=1..3, Q proj m=1..3, then out-proj chunks for
    the first tokq half as soon as its osb columns are normalized
  tail: out-proj chunks for the second tokq half

Softmax denominator: V_aug tiles are [128 tok, 8 heads, 65] (64 V dims +
ones column, memset not matmul), so PV row 64 is the denominator.
Normalize chain never blocks the PE on the DVE: den row -> SBUF copy
(DVE), PE broadcast matmul of the RAW denominator (deferred one
iteration, rotating through the feed PSUM bank), reciprocal_approx_fast
on 64 partitions, multiply.

All inputs are prefetched once into persistent SBUF tiles, DMA-ordered by
first use (V first, then Q, then K, wo last).
"""

import contextlib
import sys

import numpy as np

if "/opt/trn_rl_repo" not in sys.path:
    sys.path.insert(0, "/opt/trn_rl_repo")

import concourse.bacc as bacc
import concourse.bass as bass
import concourse.mybir as mybir
import concourse.tile as tile
from concourse.bass_utils import run_bass_kernel_spmd

FP32 = mybir.dt.float32
FP16 = mybir.dt.float16

B, NQ, NK = 4, 1024, 2048
QD, KD = 1024, 768
H, D = 16, 64
E = H * D  # 1024 total embed dim
G = 8  # heads per core
GO = G * D  # 512 out dims per core
DA = D + 1  # 65: head dim + denominator column
SCALE = 1.0 / 8.0

# test.py hooks
TRACE = False
TRACE_KWARGS = {}
LAST_RESULT = None


def _mm(nc, out, lhsT, rhs, start, stop):
    nc.tensor.matmul(out, lhsT, rhs, start=start, stop=stop)


def build_program():
    nc = bacc.Bacc()

    qT = nc.declare_dram_parameter("qT", [QD, NQ], FP16, isOutput=False)
    kT = nc.declare_dram_parameter("kT", [KD, NK], FP16, isOutput=False)
    vT = nc.declare_dram_parameter("vT", [KD, NK], FP16, isOutput=False)
    wq = nc.declare_dram_parameter("wq", [QD, GO], FP16, isOutput=False)
    wk = nc.declare_dram_parameter("wk", [KD, GO], FP16, isOutput=False)
    wv = nc.declare_dram_parameter("wv", [KD, GO], FP16, isOutput=False)
    wo = nc.declare_dram_parameter("wo", [GO, E], FP16, isOutput=False)
    vbias = nc.declare_dram_parameter("vbias", [128, GO], FP32, isOutput=False)
    bq = nc.declare_dram_parameter("bq", [128, 4], FP32, isOutput=False)
    bk = nc.declare_dram_parameter("bk", [128, 4], FP32, isOutput=False)
    out = nc.declare_dram_parameter("out", [NQ, E], FP32, isOutput=True)

    with (
        nc.allow_low_precision("fp16 attention activations; validated vs oracle"),
        tile.TileContext(nc) as tc,
    ):
        with contextlib.ExitStack() as _st:

            def _pool(name, bufs=1, **kw):
                return _st.enter_context(tc.tile_pool(name=name, bufs=bufs, **kw))

            consts = _pool("consts")
            wo_p = _pool("wo_p")
            wq_p = _pool("wq_p")
            wk_p = _pool("wk_p")
            wv_p = _pool("wv_p")
            qin_p = _pool("qin_p")
            kin_p = _pool("kin_p")
            vin_p = _pool("vin_p")
            qt_p = _pool("qt_p")
            kt_p = _pool("kt_p")
            va_p = _pool("va_p")
            osb_p = _pool("osb_p")
            ys_p = _pool("ys_p", bufs=4)
            den_p = _pool("den_p", bufs=2)
            rc_p = _pool("rc_p", bufs=2)
            bq_sb = consts.tile([128, 4], FP32)
            bk_sb = consts.tile([128, 4], FP32)
            ones_sb = consts.tile([1, 64], FP16)
            nc.vector.memset(ones_sb[:], 1.0)
            vbias_sb = consts.tile([128, 8, 64], FP32)

            wo_sb = wo_p.tile([128, 4, E], FP16, name="wo")
            wq_sb = wq_p.tile([128, 8, GO], FP16, name="wq")
            wk_sb = wk_p.tile([128, 6, GO], FP16, name="wk")
            wv_sb = wv_p.tile([128, 6, GO], FP16, name="wv")
            qin = qin_p.tile([128, 8, NQ], FP16, name="qin")
            kin = kin_p.tile([128, 6, NK], FP16, name="kin")
            vin = vin_p.tile([128, 6, NK], FP16, name="vin")

            # Persistent activation tiles.
            qt_sb = [
                [qt_p.tile([128, 512], FP16, name=f"qt{m}_{n}") for n in range(2)]
                for m in range(4)
            ]
            # Kt zero-padded per head-half (full-128-partition lhsT keeps the
            # PE at 1 cyc/col; K=64 runs 2.5x slower on real HW).
            ktz = [
                [
                    [
                        kt_p.tile([128, 512], FP16, name=f"ktz{m}_{hl}_{c}")
                        for c in range(4)
                    ]
                    for hl in range(2)
                ]
                for m in range(4)
            ]
            for m in range(4):
                for c in range(4):
                    nc.vector.memset(ktz[m][0][c][64:128, :], 0.0)
                    nc.vector.memset(ktz[m][1][c][0:64, :], 0.0)
            # V_aug[t]: [128 tokk, 8 heads, 65]
            va_sb = [va_p.tile([128, 8, DA], FP16, name=f"va{t}") for t in range(16)]
            for t in range(16):
                nc.vector.memset(va_sb[t][:, :, 64:65], 1.0)
            # O^T (normalized) [concat dim 512 -> 4 tiles of 128, tokq 1024]
            osb = [osb_p.tile([128, NQ], FP16, name=f"osb{t}") for t in range(4)]

            # ---- DMA prefetch: one descriptor per tensor, ordered by
            # first use. A descriptor occupies its issuing queue for the
            # whole transfer, so the V stream (gates phase D) rides Sync
            # while Q/K/weights ride GpSimd, which has no other work (unlike
            # Scalar, where the first exp would queue behind transfers).
            def blk(dram, g, t):
                return dram[:, :].rearrange("(g p) t -> p g t", p=128)

            nc.sync.dma_start(wv_sb[:, :, :], blk(wv, 6, GO))
            nc.sync.dma_start(
                vin[:, :, 0:512],
                vT[:, 0:512].rearrange("(g p) t -> p g t", p=128),
            )
            nc.sync.dma_start(
                vin[:, :, 512:1024],
                vT[:, 512:1024].rearrange("(g p) t -> p g t", p=128),
            )
            nc.sync.dma_start(vbias_sb[:, :, :], vbias[:, :])
            nc.gpsimd.dma_start(wq_sb[:, :, :], blk(wq, 8, GO))
            nc.gpsimd.dma_start(
                qin[:, 0:4, :],
                qT[0:512, :].rearrange("(g p) t -> p g t", p=128),
            )
            nc.sync.dma_start(
                vin[:, :, 1024:2048],
                vT[:, 1024:2048].rearrange("(g p) t -> p g t", p=128),
            )
            nc.gpsimd.dma_start(
                qin[:, 4:8, :],
                qT[512:1024, :].rearrange("(g p) t -> p g t", p=128),
            )
            nc.gpsimd.dma_start(wk_sb[:, :, :], blk(wk, 6, GO))
            nc.gpsimd.dma_start(
                kin[:, :, 0:1024],
                kT[:, 0:1024].rearrange("(g p) t -> p g t", p=128),
            )
            nc.gpsimd.dma_start(bq_sb[:], bq[:, :])
            nc.gpsimd.dma_start(bk_sb[:], bk[:, :])
            nc.gpsimd.dma_start(
                kin[:, :, 1024:2048],
                kT[:, 1024:2048].rearrange("(g p) t -> p g t", p=128),
            )
            nc.gpsimd.dma_start(wo_sb[:, :, :], blk(wo, 4, E))

            # ---- Phase D first: V_aug gates every PV chunk ----
            with tc.tile_pool(name="psD", bufs=2, space="PSUM") as psD:
                for tb in range(4):
                    psv = [
                        psD.tile([128, 8, 64], FP32, name=f"psv{t2}")
                        for t2 in range(4)
                    ]
                    for kk in range(6):
                        for t2 in range(4):
                            c0 = tb * 512 + t2 * 128
                            _mm(
                                nc,
                                psv[t2][:, :, :],
                                vin[:, kk, c0 : c0 + 128],
                                wv_sb[:, kk, :],
                                start=(kk == 0),
                                stop=(kk == 5),
                            )
                    for t2 in range(4):
                        nc.vector.tensor_add(
                            va_sb[tb * 4 + t2][:, :, 0:64],
                            psv[t2][:, :, :],
                            vbias_sb[:, :, :],
                        )

            # ---- Projection chunk emitters (prologue for m=0, feed for m>0) --
            def q_chunk(ps_pool, m, n):
                psq = ps_pool.tile([128, 512], FP32, name="pspro")
                for kk in range(8):
                    _mm(
                        nc,
                        psq[:],
                        wq_sb[:, kk, m * 128 : (m + 1) * 128],
                        qin[:, kk, n * 512 : (n + 1) * 512],
                        start=(kk == 0),
                        stop=(kk == 7),
                    )
                nc.vector.tensor_scalar_add(
                    qt_sb[m][n][:], psq[:], bq_sb[:, m : m + 1]
                )

            def k_chunk(ps_pool, m, c):
                psk = ps_pool.tile([128, 512], FP32, name="pspro")
                for kk in range(6):
                    _mm(
                        nc,
                        psk[:],
                        wk_sb[:, kk, m * 128 : (m + 1) * 128],
                        kin[:, kk, c * 512 : (c + 1) * 512],
                        start=(kk == 0),
                        stop=(kk == 5),
                    )
                nc.vector.tensor_scalar_add(
                    ktz[m][0][c][0:64, :], psk[0:64, :], bk_sb[0:64, m : m + 1]
                )
                nc.vector.tensor_scalar_add(
                    ktz[m][1][c][64:128, :], psk[64:128, :], bk_sb[64:128, m : m + 1]
                )

            # ---- Prologue: Q proj m=0, K proj m=0 ----
            with tc.tile_pool(name="psPro", bufs=4, space="PSUM") as psPro:
                for n in range(2):
                    q_chunk(psPro, 0, n)
                for c in range(4):
                    k_chunk(psPro, 0, c)

            # ---- Phase E + interleaved feed ----
            with contextlib.ExitStack() as _est:
                otp = _est.enter_context(tc.tile_pool(name="otp", bufs=2, space="PSUM"))
                stp = _est.enter_context(tc.tile_pool(name="stp", bufs=2, space="PSUM"))
                pfeed = _est.enter_context(
                    tc.tile_pool(name="pfeed", bufs=2, space="PSUM")
                )
                p_p = _est.enter_context(tc.tile_pool(name="p_p", bufs=6))
                # Feed: single-matmul emitters, popped between score chunks.
                feed = []
                for m in range(1, 4):
                    for c in range(4):
                        for kk in range(6):
                            feed.append(("k", m, c, kk))
                    for kk in range(8):
                        feed.append(("q", m, 0, kk))
                for m in range(1, 4):
                    for kk in range(8):
                        feed.append(("q", m, 1, kk))
                kdone = [4, 0, 0, 0]
                qdone = [[True, True], [False, False], [False, False], [False, False]]
                fpsq = [None]
                fpsk = [None]
                fcool = [0]

                def pump(budget):
                    while budget > 0 and feed:
                        if feed[0][0] == "f" and budget < 10**8:
                            if fcool[0] > 0:
                                fcool[0] -= 1
                                return
                            fcool[0] = 5
                        item = feed.pop(0)
                        if item[0] == "q":
                            _, m, n, kk = item
                            if kk == 0:
                                fpsq[0] = pfeed.tile([128, 512], FP32, name="pfq")
                            _mm(
                                nc,
                                fpsq[0][:],
                                wq_sb[:, kk, m * 128 : (m + 1) * 128],
                                qin[:, kk, n * 512 : (n + 1) * 512],
                                start=(kk == 0),
                                stop=(kk == 7),
                            )
                            if kk == 7:
                                nc.vector.tensor_scalar_add(
                                    qt_sb[m][n][:], fpsq[0][:], bq_sb[:, m : m + 1]
                                )
                                qdone[m][n] = True
                        elif item[0] == "k":
                            _, m, c, kk = item
                            if kk == 0:
                                fpsk[0] = pfeed.tile([128, 512], FP32, name="pfq")
                            _mm(
                                nc,
                                fpsk[0][:],
                                wk_sb[:, kk, m * 128 : (m + 1) * 128],
                                kin[:, kk, c * 512 : (c + 1) * 512],
                                start=(kk == 0),
                                stop=(kk == 5),
                            )
                            if kk == 5:
                                nc.vector.tensor_scalar_add(
                                    ktz[m][0][c][0:64, :],
                                    psk_view(fpsk[0], 0),
                                    bk_sb[0:64, m : m + 1],
                                )
                                nc.vector.tensor_scalar_add(
                                    ktz[m][1][c][64:128, :],
                                    psk_view(fpsk[0], 1),
                                    bk_sb[64:128, m : m + 1],
                                )
                                kdone[m] += 1
                        else:  # out-proj chunk: 4 mms + copy + dma
                            _, m, n = item
                            psy = pfeed.tile([128, 512], FP32, name="pfq")
                            for kt in range(4):
                                _mm(
                                    nc,
                                    psy[:],
                                    osb[kt][:, m * 128 : (m + 1) * 128],
                                    wo_sb[:, kt, n * 512 : (n + 1) * 512],
                                    start=(kt == 0),
                                    stop=(kt == 3),
                                )
                            ys = ys_p.tile([128, 512], FP32, name="ys")
                            nc.vector.tensor_copy(ys[:], psy[:])
                            nc.sync.dma_start(
                                out[m * 128 : (m + 1) * 128, n * 512 : (n + 1) * 512],
                                ys[:],
                            )
                            budget -= 3
                        budget -= 1

                def psk_view(t, hl):
                    return t[hl * 64 : hl * 64 + 64, :]

                iters = [(h, n) for n in range(2) for h in range(G)]
                pending = []
                ot_cur = [None]

                def flush_pending(ps_pool=None):
                    ot_p, den_t, mt_p, po_p, n_p = pending.pop(0)
                    # Broadcast the raw denominator across 64 partitions.
                    # Shares the single feed PSUM bank (base partition 0 —
                    # base 64 miscompiles to a bogus col_grp on HW).
                    bc_t = (ps_pool or pfeed).tile([128, 512], FP32, name="pfq")
                    nc.tensor.matmul(
                        bc_t[:64, :], ones_sb[:, :], den_t[:],
                        start=True, stop=True,
                    )
                    rc_t = rc_p.tile([64, 512], FP32, name="rc")
                    nc.vector.reciprocal_approx_fast(rc_t[:], bc_t[:64, :])
                    nc.vector.tensor_mul(
                        osb[mt_p][po_p : po_p + 64, n_p * 512 : (n_p + 1) * 512],
                        ot_p[:64, :],
                        rc_t[:],
                    )

                def do_pv(pit, pg, pp):
                    ph, pn = iters[pit]
                    if pg == 0:
                        ot_cur[0] = otp.tile([128, 512], FP32, name="ot")
                    ot = ot_cur[0]
                    for j in range(2):
                        t = 2 * pg + j
                        _mm(
                            nc,
                            ot[:65, :],
                            va_sb[t][:, ph : ph + 1, :],
                            pp[:, j * 512 : (j + 1) * 512],
                            start=(pg == 0 and j == 0),
                            stop=(pg == 7 and j == 1),
                        )
                    if pg == 4 and pending:
                        # flush the previous iteration mid-iteration so its
                        # ot bank frees in time for the 2-buffer rotation
                        flush_pending()
                    if pg == 7:
                        den_t = den_p.tile([1, 512], FP16, name="den")
                        nc.vector.tensor_copy(den_t[:], ot[64:65, :])
                        pending.append((ot, den_t, ph // 2, (ph % 2) * 64, pn))

                queue = []
                for it in range(16):
                    h, n = iters[it]
                    mt, hl = h // 2, h % 2
                    while feed and not (kdone[mt] == 4 and qdone[mt][n]):
                        pump(4)
                    if it == 10:
                        # first tokq half fully normalized: flush(n0,h7) is
                        # emitted during it9, so feeding from it10 is safe
                        for fm in range(4):
                            for fn in range(2):
                                feed.append(("f", fm, fn))
                    for g2 in range(8):
                        # PV for the chunk two back goes first: it gives the
                        # exp feeding the two-ahead score chunk extra slack.
                        if len(queue) >= 2:
                            do_pv(*queue.pop(0))
                        st2 = stp.tile([128, 1024], FP32, name="st")
                        for j in range(2):
                            kt = 2 * g2 + j
                            _mm(
                                nc,
                                st2[:, j * 512 : (j + 1) * 512],
                                ktz[mt][hl][kt // 4][
                                    :, (kt % 4) * 128 : (kt % 4 + 1) * 128
                                ],
                                qt_sb[mt][n][:],
                                start=True,
                                stop=True,
                            )
                        p2 = p_p.tile([128, 1024], FP16, name="p")
                        nc.scalar.activation(
                            p2[:],
                            st2[:],
                            mybir.ActivationFunctionType.Exp,
                            bias=0.0,
                            scale=SCALE,
                        )
                        queue.append((it, g2, p2))
                        pump(2)
                while queue:
                    do_pv(*queue.pop(0))
                while pending:
                    flush_pending()
                pump(10**9)

            # ---- Tail: remaining out-proj chunks ----
            with (
                tc.tile_pool(name="psF", bufs=4, space="PSUM") as psF,
                tc.tile_pool(name="ysT", bufs=4) as ysT,
            ):
                for m in range(4, 8):
                    for n in range(2):
                        psy = psF.tile([128, 512], FP32, name="psy")
                        for kt in range(4):
                            _mm(
                                nc,
                                psy[:],
                                osb[kt][:, m * 128 : (m + 1) * 128],
                                wo_sb[:, kt, n * 512 : (n + 1) * 512],
                                start=(kt == 0),
                                stop=(kt == 3),
                            )
                        ys = ysT.tile([128, 512], FP32, name="ys")
                        if n == 0:
                            # Scalar engine is idle in the tail; split the
                            # staging copies across ACT and DVE
                            nc.scalar.activation(
                                ys[:], psy[:],
                                mybir.ActivationFunctionType.Copy,
                            )
                        else:
                            nc.vector.tensor_copy(ys[:], psy[:])
                        nc.sync.dma_start(
                            out[m * 128 : (m + 1) * 128, n * 512 : (n + 1) * 512],
                            ys[:],
                        )

    nc.finalize()
    return nc


def kernel(**inputs):
    global LAST_RESULT
    arrs = {k: np.asarray(v, dtype=np.float32) for k, v in inputs.items()}
    query, key, value = arrs["query"], arrs["key"], arrs["value"]
    Wq, bq_, Wk, bk_ = arrs["Wq"], arrs["bq"], arrs["Wk"], arrs["bk"]
    Wv, bv_, Wo, bo_ = arrs["Wv"], arrs["bv"], arrs["Wo"], arrs["bo"]

    nc = build_program()

    qTb = [np.ascontiguousarray(query[b].T.astype(np.float16)) for b in range(B)]
    kTb = [np.ascontiguousarray(key[b].T.astype(np.float16)) for b in range(B)]
    vTb = [np.ascontiguousarray(value[b].T.astype(np.float16)) for b in range(B)]

    per_group = []
    for g in range(2):
        gs = slice(g * GO, (g + 1) * GO)
        wq_m = np.ascontiguousarray(Wq[gs, :].T.astype(np.float16))
        wk_m = np.ascontiguousarray(Wk[gs, :].T.astype(np.float16))
        wv_m = np.ascontiguousarray(Wv[gs, :].T.astype(np.float16))
        vb_row = bv_[gs].astype(np.float32)  # head-major [8*64]
        vbias_m = np.ascontiguousarray(np.tile(vb_row, (128, 1)).astype(np.float32))
        wo_m = np.ascontiguousarray(Wo[:, gs].T.astype(np.float16))
        bq_m = np.ascontiguousarray(bq_[gs].reshape(4, 128).T)
        bk_m = np.ascontiguousarray(bk_[gs].reshape(4, 128).T)
        per_group.append(
            {
                "wq": wq_m,
                "wk": wk_m,
                "wv": wv_m,
                "wo": wo_m,
                "vbias": vbias_m,
                "bq": bq_m,
                "bk": bk_m,
            }
        )

    in_maps = []
    for c in range(8):
        b, g = c // 2, c % 2
        m = {"qT": qTb[b], "kT": kTb[b], "vT": vTb[b]}
        m.update(per_group[g])
        in_maps.append(m)

    res = run_bass_kernel_spmd(
        nc, in_maps, list(range(8)), trace=TRACE, **(TRACE_KWARGS if TRACE else {})
    )
    LAST_RESULT = res

    outs = res.results
    Y = np.empty((B, NQ, E), np.float32)
    for b in range(B):
        Y[b] = outs[2 * b]["out"] + outs[2 * b + 1]["out"] + bo_[None, :]
    return Y
